# revision 25
# baseline (speedup 1.0000x reference)
"""Trainium2 Bass kernel for nn_AttnConvKernel (conv3x3 x2 -> unfold -> gram -> softmax).

Sharding: 8 cores = 4 batch samples x 2 H-halves. Each core computes both convs
for its half in a single fused matmul pass that directly produces the
[positions, channels] (transposed) layout needed by the attention contraction:
the 3x3-shifted x window is the stationary operand and [W1*scale | W2]
(128x384) is the moving operand, in float32r. Attention logits accumulate in
PSUM across the 64 patch-row tiles; a pairwise AllReduce sums the two
half-sample partials; softmax over (cin, 3x3) runs on-device.

Performance notes (HW-measured via in-NEFF reps amplification):
- The tile loop is 917us/pass = ~100% PE occupancy at full fp32r rate
  (891us of pure MACs) -- the algorithmic floor for direct conv. Winograd
  and low-precision variants were analyzed and rejected (transform cost on
  DVE/ACT, resp. softmax logit precision). Correlation-algebra rewrites
  (expand conv(s) into x-autocorrelation grams contracted with weights)
  were costed at ~1.27x (single-expansion) / ~1.4x (double-expansion) PE
  reduction but rejected on transpose overhead + boundary-term complexity.
- Sustained passes (reps>=2 in one NEFF) run ~1130us, not 917us; a 5x
  DMA-traffic cut in rep2 did NOT recover it (not DMA-bound; likely PE
  p-state/power). With reps=1 production each launch gets the fast pass.
- The tail collective chain costs ~56us per launch, mostly first-use
  init/sync (steady-state CC is ~9-19us). cc_warm issues a tiny dummy
  AllReduce at kernel start (hidden under the conv loop) to prewarm the
  channel: -10..-51us measured. Splitting the logit CC in two phases
  added fixed cost, no reliable win. fast_dispatch_compile (C++ dispatch
  path) made no difference -- launches already pipeline; the zero-output
  operands were dead and are dropped.
- Startup DMAs are issued as [w-tap0, x-tile0 rows 0:3, rows 3:5,
  w-taps 1:9] so the first conv matmul starts ~4us in instead of ~10us
  (HWDGE serializes dma_starts in issue order).
- Tail trims: chunked post-collective logit loads (transposes overlap the
  load) and per-half output DMA (h=0 writeback overlaps h=1 softmax).
"""

import numpy as np

B, CIN, COUT = 4, 128, 256
H = W = 384
WP = W // 3              # 128 patch columns
HALF_ROWS = H // 2       # 192
TILES = HALF_ROWS // 3   # 64 patch-rows per core
CH = CIN + COUT          # 384 fused output channels
NCORES = 8
SCALE = 1.0 / np.sqrt(CIN * 9)

_compiled = None
_runners = {}


def _build_nc(
    reps=1,
    act_split=True,
    lag=3,
    psum4=False,
    xp_bufs=None,
    act_every=2,
    rs_tail=False,
    split_w=True,
    split_x0=True,
    rs_kind="ReduceScatter",
    tail_reps=1,
    cc_skip=False,
    cc_warm=False,
    cc_split=False,
    dma_rows=5,
):
    import concourse.mybir as mybir
    import concourse.tile as tile
    from concourse import bacc
    from concourse.masks import make_identity

    f32 = mybir.dt.float32
    f32r = mybir.dt.float32r

    nc = bacc.Bacc(target_bir_lowering=False, num_devices=NCORES)
    # x_half: rows [3t .. 3t+4] cover patch-row t with halo; cols pre-padded.
    x_half = nc.dram_tensor(
        "x_half", [CIN, HALF_ROWS + 2, W + 2], f32, kind="ExternalInput"
    )
    wcat = nc.dram_tensor("wcat", [CIN, 9, CH], f32, kind="ExternalInput")
    if rs_tail:
        # each core outputs softmax rows for one COUT-half (ReduceScatter rank)
        out_t = nc.dram_tensor("out", [COUT // 2, CIN, 9], f32, kind="ExternalOutput")
        # cc_in[h, c, k, o'] = logits for o = h*128 + o'; ReduceScatter over the
        # pair sums and hands chunk h to pair-rank h.
        cc_in = nc.dram_tensor("cc_in", [2, CIN, 9, COUT // 2], f32)
        if rs_kind == "ReduceScatter":
            cc_out = nc.dram_tensor("cc_out", [CIN, 9, COUT // 2], f32)
        else:
            cc_out = nc.dram_tensor("cc_out", [2, CIN, 9, COUT // 2], f32)
    else:
        out_t = nc.dram_tensor("out", [COUT, CIN, 9], f32, kind="ExternalOutput")
        if cc_split:
            # two-phase collective: banks 0-1 (k0..3) ship while the tile
            # loop still flushes; k4..8 follow
            cc_in = nc.dram_tensor("cc_in", [CIN, 4 * COUT], f32)
            cc_out = nc.dram_tensor("cc_out", [CIN, 4 * COUT], f32)
            cc_in_b = nc.dram_tensor("cc_in_b", [CIN, 5 * COUT], f32)
            cc_out_b = nc.dram_tensor("cc_out_b", [CIN, 5 * COUT], f32)
        else:
            cc_in = nc.dram_tensor("cc_in", [CIN, 9 * COUT], f32)
            cc_out = nc.dram_tensor("cc_out", [CIN, 9 * COUT], f32)
    if cc_warm:
        ccw_in = nc.dram_tensor("ccw_in", [1, 8], f32)
        ccw_out = nc.dram_tensor("ccw_out", [1, 8], f32)

    with tile.TileContext(nc) as tc:
        with (
            tc.tile_pool(name="xp", bufs=xp_bufs or (4 if psum4 else 3)) as xp,
            tc.tile_pool(name="wp", bufs=1) as wp,
            tc.tile_pool(
                name="yp", bufs=(9 if psum4 else 6) if lag <= 2 else lag + 5
            ) as yp,
            tc.tile_pool(name="sp", bufs=1) as sp,
            tc.tile_pool(name="pc", bufs=4 if psum4 else 3, space="PSUM") as pc,
            tc.tile_pool(name="pa", bufs=1, space="PSUM") as pa,
        ):
            w_sb = wp.tile([CIN, 9, CH], f32r)
            x0 = xp.tile([CIN, 5, W + 2], f32r, name="xt") if split_x0 else None
            if split_w:
                # startup order: tap0 weights + tile-0 x rows first, so the
                # first conv matmul starts ~4us in instead of ~10us (HWDGE
                # serializes dma_starts in issue order)
                nc.sync.dma_start(
                    out=w_sb[:, 0, :], in_=wcat[:, 0, :].bitcast(f32r)
                )
                if split_x0:
                    # row 0 alone unblocks the very first matmul (k=0, tap=0)
                    nc.sync.dma_start(
                        out=x0[:, 0:1, :], in_=x_half[:, 0:1, :].bitcast(f32r)
                    )
                    nc.sync.dma_start(
                        out=x0[:, 1:3, :], in_=x_half[:, 1:3, :].bitcast(f32r)
                    )
                    nc.sync.dma_start(
                        out=x0[:, 3:5, :], in_=x_half[:, 3:5, :].bitcast(f32r)
                    )
                for tap in range(1, 9):
                    nc.sync.dma_start(
                        out=w_sb[:, tap, :], in_=wcat[:, tap, :].bitcast(f32r)
                    )
            else:
                nc.sync.dma_start(out=w_sb, in_=wcat[:, :, :].bitcast(f32r))
                if split_x0:
                    nc.sync.dma_start(
                        out=x0[:, 0:3, :], in_=x_half[:, 0:3, :].bitcast(f32r)
                    )
                    nc.sync.dma_start(
                        out=x0[:, 3:5, :], in_=x_half[:, 3:5, :].bitcast(f32r)
                    )

            if cc_warm:
                # warm the collective channel early so the tail CC doesn't
                # pay first-use init/sync: tiny AllReduce overlapped with
                # the conv loop (content irrelevant, uses loaded w data)
                ccw_sb = sp.tile([1, 8], f32)
                nc.vector.tensor_copy(out=ccw_sb, in_=w_sb[0:1, 0, 0:8].bitcast(f32))
                nc.sync.dma_start(out=ccw_in[:, :], in_=ccw_sb)
                nc.gpsimd.collective_compute(
                    "AllReduce",
                    mybir.AluOpType.add,
                    replica_groups=[[0, 1], [2, 3], [4, 5], [6, 7]],
                    ins=[ccw_in.ap().opt()],
                    outs=[ccw_out.ap().opt()],
                )

            # persistent attn logit accumulators: 2 k's per PSUM bank
            attn_ps = [
                pa.tile([CIN, 2, COUT], f32, tag=f"attn{i}", name=f"attn{i}")
                for i in range(4)
            ]
            if psum4:
                # k=8 accumulates in SBUF (frees a PSUM bank for the conv pool)
                acc8 = sp.tile([CIN, COUT], f32)
            else:
                attn_ps.append(
                    pa.tile([CIN, 1, COUT], f32, tag="attn4", name="attn4")
                )

            def attn_mm(k, yk, t):
                if psum4 and k == 8:
                    ps8 = pc.tile([CIN, COUT], f32, tag="conv", name="ps8")
                    nc.tensor.matmul(
                        ps8, yk[:, 0:CIN], yk[:, CIN:CH], start=True, stop=True
                    )
                    nc.vector.tensor_add(acc8, acc8, ps8)
                    return
                # start=True clears has_written for the WHOLE bank, so only the
                # first k of each 2-k bank may set it (at t=0). The second k's
                # first matmul overwrites via per-element has_written bits.
                nc.tensor.matmul(
                    attn_ps[k // 2][:, k % 2, :],
                    yk[:, 0:CIN],
                    yk[:, CIN:CH],
                    start=(t == 0 and k % 2 == 0),
                    stop=(t == TILES - 1),
                    skip_group_check=True,
                )

            for rep in range(reps):
                if psum4:
                    nc.vector.memset(acc8, 0.0)
                for t in range(TILES):
                    if split_x0 and rep == 0 and t == 0:
                        xt = x0  # pre-loaded row-chunked before the weights
                    else:
                        xt = xp.tile([CIN, 5, W + 2], f32r, name="xt")
                        nr = 5 if rep == 0 else dma_rows
                        # dma_rows<5 on rep>0: timing probe only -- the tile
                        # pool buffer holds stale (finite) rows beyond nr
                        nc.sync.dma_start(
                            out=xt[:, 0:nr, :],
                            in_=x_half[:, 3 * t : 3 * t + nr, :].bitcast(f32r),
                        )
                    yks = []
                    for k in range(9):
                        kh, kw = divmod(k, 3)
                        ps = pc.tile([WP, CH], f32, tag="conv", name="ps")
                        for tap in range(9):
                            dh1, dw1 = divmod(tap, 3)
                            s = kw + dw1
                            lhsT = xt[:, kh + dh1, s : s + 3 * WP - 2 : 3]
                            nc.tensor.matmul(
                                ps,
                                lhsT,
                                w_sb[:, tap, :],
                                start=(tap == 0),
                                stop=(tap == 8),
                            )
                        yk = yp.tile([WP, CH], f32r, tag="y", name="yk")
                        if act_split and k % act_every == 0:
                            nc.scalar.copy(out=yk, in_=ps)
                        else:
                            nc.vector.tensor_copy(out=yk, in_=ps)
                        yks.append(yk)
                        # lag attn matmuls behind the drain copies so PE never
                        # waits on a copy
                        if k >= lag:
                            attn_mm(k - lag, yks[k - lag], t)
                    for k in range(9 - lag, 9):
                        attn_mm(k, yks[k], t)

            for _tr in range(tail_reps):
                _tail(
                    nc, tc, mybir, sp, pc, make_identity,
                    rs_tail, rs_kind, psum4,
                    attn_ps, acc8 if psum4 else None,
                    cc_in, cc_out, out_t, f32,
                    cc_skip=cc_skip,
                    cc_split=cc_split,
                    cc_b=(cc_in_b, cc_out_b) if cc_split else None,
                )
    nc.compile()
    return nc


def _tail(
    nc, tc, mybir, sp, pc, make_identity,
    rs_tail, rs_kind, psum4, attn_ps, acc8, cc_in, cc_out, out_t, f32,
    cc_skip=False,
    cc_split=False,
    cc_b=None,
):
    if cc_split:
        _tail_split(
            nc, tc, mybir, sp, pc, make_identity,
            psum4, attn_ps, acc8, cc_in, cc_out, cc_b, out_t, f32,
        )
        return
    if True:
        if True:
            if rs_tail:
                # ---- tail: ReduceScatter pair halves, softmax o-half ----
                lg = sp.tile([CIN, 9, COUT], f32)
                for i in range(4):
                    nc.vector.tensor_copy(
                        out=lg[:, 2 * i : 2 * i + 2, :], in_=attn_ps[i]
                    )
                if psum4:
                    nc.vector.tensor_copy(out=lg[:, 8, :], in_=acc8)
                else:
                    nc.vector.tensor_copy(out=lg[:, 8, :], in_=attn_ps[4][:, 0, :])
                for h in range(2):
                    nc.sync.dma_start(
                        out=cc_in[h], in_=lg[:, :, h * 128 : (h + 1) * 128]
                    )
                groups = [[0, 1], [2, 3], [4, 5], [6, 7]]
                lgs = sp.tile([CIN, 9, COUT // 2], f32)
                if rs_kind == "ReduceScatter":
                    nc.gpsimd.collective_compute(
                        "ReduceScatter",
                        mybir.AluOpType.add,
                        replica_groups=groups,
                        ins=[cc_in.ap().opt()],
                        outs=[cc_out.ap().opt()],
                    )
                    nc.sync.dma_start(out=lgs, in_=cc_out[:, :, :])
                elif rs_kind == "AllToAll":
                    # pure pair exchange: out[j] = chunk<rank> of member j;
                    # adding both gives this rank's reduced o-half
                    nc.gpsimd.collective_compute(
                        "AllToAll",
                        mybir.AluOpType.bypass,
                        replica_groups=groups,
                        ins=[cc_in.ap().opt()],
                        outs=[cc_out.ap().opt()],
                    )
                    la = sp.tile([CIN, 9, COUT // 2], f32)
                    lb = sp.tile([CIN, 9, COUT // 2], f32)
                    nc.sync.dma_start(out=la, in_=cc_out[0])
                    nc.sync.dma_start(out=lb, in_=cc_out[1])
                    nc.vector.tensor_add(lgs, la, lb)
                else:
                    nc.gpsimd.collective_compute(
                        "AllReduce",
                        mybir.AluOpType.add,
                        replica_groups=groups,
                        ins=[cc_in.ap().opt()],
                        outs=[cc_out.ap().opt()],
                    )
                    # timing-only variant: rank-agnostic half read
                    nc.sync.dma_start(out=lgs, in_=cc_out[0])

                ident = sp.tile([128, 128], f32)
                make_identity(nc, ident)
                soft = sp.tile([128, CIN, 9], f32)
                for k in range(9):
                    tp = pc.tile([128, 128], f32, tag="conv")
                    nc.tensor.transpose(out=tp, in_=lgs[:, k, :], identity=ident)
                    nc.vector.tensor_copy(out=soft[:, :, k], in_=tp)

                mx = sp.tile([128, 1], f32)
                nmx = sp.tile([128, 1], f32)
                sm = sp.tile([128, 1], f32)
                rs = sp.tile([128, 1], f32)
                nc.vector.reduce_max(out=mx, in_=soft, axis=mybir.AxisListType.XY)
                nc.scalar.mul(out=nmx, in_=mx, mul=-1.0)
                nc.scalar.activation(
                    out=soft,
                    in_=soft,
                    func=mybir.ActivationFunctionType.Exp,
                    bias=nmx,
                    scale=1.0,
                    accum_out=sm,
                )
                nc.vector.reciprocal(out=rs, in_=sm)
                nc.vector.tensor_scalar_mul(soft, soft, rs)

                nc.sync.dma_start(out=out_t[:, :, :], in_=soft)
            else:
                # ---- tail: AllReduce, softmax both halves (legacy) ----
                # per-bank copy+store: bank i only depends on attn_mm(2i+1),
                # so its drain overlaps the remaining attn matmuls of tile 63
                lg = sp.tile([CIN, 9 * COUT], f32)
                for i in range(4):
                    nc.vector.tensor_copy(
                        out=lg[:, i * 512 : (i + 1) * 512], in_=attn_ps[i]
                    )
                    nc.sync.dma_start(
                        out=cc_in[:, i * 512 : (i + 1) * 512],
                        in_=lg[:, i * 512 : (i + 1) * 512],
                    )
                if psum4:
                    nc.vector.tensor_copy(out=lg[:, 2048:2304], in_=acc8)
                else:
                    nc.vector.tensor_copy(
                        out=lg[:, 2048:2304], in_=attn_ps[4][:, 0, :]
                    )
                nc.sync.dma_start(out=cc_in[:, 2048:2304], in_=lg[:, 2048:2304])
                if cc_skip:
                    # timing probe: no collective; read own partials (wrong
                    # values, same dataflow shape)
                    cc_src = cc_in
                else:
                    nc.gpsimd.collective_compute(
                        "AllReduce",
                        mybir.AluOpType.add,
                        replica_groups=[[0, 1], [2, 3], [4, 5], [6, 7]],
                        ins=[cc_in.ap().opt()],
                        outs=[cc_out.ap().opt()],
                    )
                    cc_src = cc_out
                lgs = sp.tile([CIN, 9, COUT], f32)
                cc3 = cc_src[:, :].rearrange("p (k o) -> p k o", k=9)
                # chunked load: transposes on k 0-2 start before k 3-8 land
                for k0 in range(0, 9, 3):
                    nc.sync.dma_start(
                        out=lgs[:, k0 : k0 + 3, :], in_=cc3[:, k0 : k0 + 3, :]
                    )

                ident = sp.tile([128, 128], f32)
                make_identity(nc, ident)
                soft = sp.tile([128, 2, CIN, 9], f32)
                for h in range(2):
                    for k in range(9):
                        tp = pc.tile([128, 128], f32, tag="conv")
                        nc.tensor.transpose(
                            out=tp,
                            in_=lgs[:, k, h * 128 : (h + 1) * 128],
                            identity=ident,
                        )
                        nc.vector.tensor_copy(out=soft[:, h, :, k], in_=tp)

                mx = sp.tile([128, 2], f32)
                nmx = sp.tile([128, 2], f32)
                sm = sp.tile([128, 2], f32)
                rs = sp.tile([128, 2], f32)
                for h in range(2):
                    nc.vector.reduce_max(
                        out=mx[:, h : h + 1],
                        in_=soft[:, h],
                        axis=mybir.AxisListType.XY,
                    )
                    nc.scalar.mul(
                        out=nmx[:, h : h + 1], in_=mx[:, h : h + 1], mul=-1.0
                    )
                    nc.scalar.activation(
                        out=soft[:, h],
                        in_=soft[:, h],
                        func=mybir.ActivationFunctionType.Exp,
                        bias=nmx[:, h : h + 1],
                        scale=1.0,
                        accum_out=sm[:, h : h + 1],
                    )
                    nc.vector.reciprocal(out=rs[:, h : h + 1], in_=sm[:, h : h + 1])
                    nc.vector.tensor_scalar_mul(
                        soft[:, h], soft[:, h], rs[:, h : h + 1]
                    )
                    # write this half back while the other half's softmax runs
                    nc.sync.dma_start(
                        out=out_t[:, :, :].rearrange("(h p) i k -> p h i k", h=2)[
                            :, h
                        ],
                        in_=soft[:, h],
                    )


def _tail_split(
    nc, tc, mybir, sp, pc, make_identity,
    psum4, attn_ps, acc8, cc_in, cc_out, cc_b, out_t, f32,
):
    """Two-phase collective tail: k0..3 (banks 0,1) AllReduce while the tile
    loop flushes the remaining attn matmuls; k4..8 follow; the k0..3
    transposes overlap the second collective."""
    cc_in_b, cc_out_b = cc_b
    CIN_, COUT_ = CIN, COUT
    groups = [[0, 1], [2, 3], [4, 5], [6, 7]]

    lg = sp.tile([CIN_, 9 * COUT_], f32)
    # phase a: banks 0,1 = k0..3 = cols 0:1024
    for i in range(2):
        nc.vector.tensor_copy(out=lg[:, i * 512 : (i + 1) * 512], in_=attn_ps[i])
        nc.sync.dma_start(
            out=cc_in[:, i * 512 : (i + 1) * 512],
            in_=lg[:, i * 512 : (i + 1) * 512],
        )
    nc.gpsimd.collective_compute(
        "AllReduce",
        mybir.AluOpType.add,
        replica_groups=groups,
        ins=[cc_in.ap().opt()],
        outs=[cc_out.ap().opt()],
    )
    # phase b: banks 2,3 + k8 = cols 1024:2304 -> cc_in_b cols 0:1280
    for i in range(2, 4):
        nc.vector.tensor_copy(out=lg[:, i * 512 : (i + 1) * 512], in_=attn_ps[i])
        nc.sync.dma_start(
            out=cc_in_b[:, (i - 2) * 512 : (i - 1) * 512],
            in_=lg[:, i * 512 : (i + 1) * 512],
        )
    if psum4:
        nc.vector.tensor_copy(out=lg[:, 2048:2304], in_=acc8)
    else:
        nc.vector.tensor_copy(out=lg[:, 2048:2304], in_=attn_ps[4][:, 0, :])
    nc.sync.dma_start(out=cc_in_b[:, 1024:1280], in_=lg[:, 2048:2304])
    nc.gpsimd.collective_compute(
        "AllReduce",
        mybir.AluOpType.add,
        replica_groups=groups,
        ins=[cc_in_b.ap().opt()],
        outs=[cc_out_b.ap().opt()],
    )

    lgs = sp.tile([CIN_, 9, COUT_], f32)
    cc3a = cc_out[:, :].rearrange("p (k o) -> p k o", k=4)
    cc3b = cc_out_b[:, :].rearrange("p (k o) -> p k o", k=5)
    # k0..3 load right after CC_a -> their transposes run under CC_b
    nc.sync.dma_start(out=lgs[:, 0:4, :], in_=cc3a)
    for k0 in range(4, 9, 3):
        k1 = min(k0 + 3, 9)
        nc.sync.dma_start(
            out=lgs[:, k0:k1, :], in_=cc3b[:, k0 - 4 : k1 - 4, :]
        )

    ident = sp.tile([128, 128], f32)
    make_identity(nc, ident)
    soft = sp.tile([128, 2, CIN_, 9], f32)
    # k-major so the k0..3 transposes (ready after CC_a) run under CC_b
    for k in range(9):
        for h in range(2):
            tp = pc.tile([128, 128], f32, tag="conv")
            nc.tensor.transpose(
                out=tp,
                in_=lgs[:, k, h * 128 : (h + 1) * 128],
                identity=ident,
            )
            nc.vector.tensor_copy(out=soft[:, h, :, k], in_=tp)

    mx = sp.tile([128, 2], f32)
    nmx = sp.tile([128, 2], f32)
    sm = sp.tile([128, 2], f32)
    rs = sp.tile([128, 2], f32)
    for h in range(2):
        nc.vector.reduce_max(
            out=mx[:, h : h + 1], in_=soft[:, h], axis=mybir.AxisListType.XY
        )
        nc.scalar.mul(out=nmx[:, h : h + 1], in_=mx[:, h : h + 1], mul=-1.0)
        nc.scalar.activation(
            out=soft[:, h],
            in_=soft[:, h],
            func=mybir.ActivationFunctionType.Exp,
            bias=nmx[:, h : h + 1],
            scale=1.0,
            accum_out=sm[:, h : h + 1],
        )
        nc.vector.reciprocal(out=rs[:, h : h + 1], in_=sm[:, h : h + 1])
        nc.vector.tensor_scalar_mul(soft[:, h], soft[:, h], rs[:, h : h + 1])
        nc.sync.dma_start(
            out=out_t[:, :, :].rearrange("(h p) i k -> p h i k", h=2)[:, h],
            in_=soft[:, h],
        )


def _prep_inputs(x, w1, w2):
    x = np.ascontiguousarray(np.asarray(x, dtype=np.float32))
    w1 = np.asarray(w1, dtype=np.float32)
    w2 = np.asarray(w2, dtype=np.float32)

    wcat = np.empty((CIN, 9, CH), np.float32)
    for dh in range(3):
        for dw in range(3):
            tap = dh * 3 + dw
            wcat[:, tap, :CIN] = w1[:, :, dh, dw].T * SCALE
            wcat[:, tap, CIN:] = w2[:, :, dh, dw].T

    xp = np.zeros((B, CIN, H + 2, W + 2), np.float32)
    xp[:, :, 1:-1, 1:-1] = x

    in_maps = []
    for c in range(NCORES):
        b, h = divmod(c, 2)
        xh = np.ascontiguousarray(xp[b, :, h * HALF_ROWS : h * HALF_ROWS + 194, :])
        in_maps.append({"x_half": xh, "wcat": wcat})
    return in_maps


class _Runner:
    """Compile once, execute many times with device-resident inputs."""

    def __init__(self, reps=1, fast_dispatch=True, **build_kw):
        import jax
        import concourse.mybir as mybir
        from concourse import bass2jax
        from jax.sharding import Mesh, PartitionSpec, NamedSharding
        from jax.experimental.shard_map import shard_map

        self.jax = jax
        nc = _build_nc(reps=reps, **build_kw)
        bass2jax.install_neuronx_cc_hook()

        partition_name = (
            nc.partition_id_tensor.name if nc.partition_id_tensor else None
        )
        in_names, out_names, out_avals, zero_outs = [], [], [], []
        for alloc in nc.m.functions[0].allocations:
            if not isinstance(alloc, mybir.MemoryLocationSet):
                continue
            name = alloc.memorylocations[0].name
            if alloc.kind == "ExternalInput":
                if name != partition_name:
                    in_names.append(name)
            elif alloc.kind == "ExternalOutput":
                out_names.append(name)
                shape = tuple(alloc.tensor_shape)
                dtype = mybir.dt.np(alloc.dtype)
                out_avals.append(jax.core.ShapedArray(shape, dtype))
                zero_outs.append(np.zeros(shape, dtype))
        n_params = len(in_names)
        n_outs = len(out_avals)
        all_names = list(in_names)
        if partition_name is not None:
            all_names = all_names + [partition_name]

        def _body(*args):
            operands = list(args)
            if partition_name is not None:
                operands.append(bass2jax.partition_id_tensor())
            outs = bass2jax._bass_exec_p.bind(
                *operands,
                out_avals=tuple(out_avals),
                in_names=tuple(all_names),
                out_names=tuple(out_names),
                lowering_input_output_aliases=(),
                sim_require_finite=True,
                sim_require_nnan=True,
                nc=nc,
            )
            return tuple(outs)

        devices = jax.devices()[:NCORES]
        mesh = Mesh(np.asarray(devices), ("core",))
        self.sharding = NamedSharding(mesh, PartitionSpec("core"))
        # zero-output operands are dead without donation (the NKI lowering
        # allocates ExternalOutputs itself and this kernel writes every
        # element) -- drop them from the call path entirely.
        jitted = jax.jit(
            shard_map(
                _body,
                mesh=mesh,
                in_specs=(PartitionSpec("core"),) * n_params,
                out_specs=(PartitionSpec("core"),) * n_outs,
                check_rep=False,
            ),
            keep_unused=True,
        )
        if fast_dispatch:
            # suppress BassEffect so launches take the C++ fast-dispatch
            # path and pipeline on-device instead of serializing host RTT
            # behind each execution
            in_sds = []
            for alloc in nc.m.functions[0].allocations:
                if not isinstance(alloc, mybir.MemoryLocationSet):
                    continue
                if alloc.kind != "ExternalInput":
                    continue
                name = alloc.memorylocations[0].name
                if name == partition_name:
                    continue
                shape = tuple(alloc.tensor_shape)
                dtype = mybir.dt.np(alloc.dtype)
                in_sds.append(
                    jax.ShapeDtypeStruct(
                        (NCORES * shape[0], *shape[1:]), dtype, sharding=self.sharding
                    )
                )
            self.sharded = bass2jax.fast_dispatch_compile(
                lambda: jitted.lower(*in_sds).compile()
            )
        else:
            self.sharded = jitted
        self.in_names = in_names
        self.out_names = out_names
        self.out_avals = out_avals

    def put_inputs(self, in_maps):
        concat = [
            np.concatenate([np.asarray(m[name]) for m in in_maps], axis=0)
            for name in self.in_names
        ]
        return [self.jax.device_put(a, self.sharding) for a in concat]

    def execute(self, dev_inputs, n=1, block=True):
        for _ in range(n):
            out_arrs = self.sharded(*dev_inputs)
        if block:
            self.jax.block_until_ready(out_arrs)
        return out_arrs

    def run(self, in_maps):
        out_arrs = self.execute(self.put_inputs(in_maps))
        res = []
        for c in range(NCORES):
            res.append(
                {
                    name: np.asarray(out_arrs[i]).reshape(
                        NCORES, *self.out_avals[i].shape
                    )[c]
                    for i, name in enumerate(self.out_names)
                }
            )
        return res


def get_runner(reps=1, **build_kw):
    key = (reps, tuple(sorted(build_kw.items())))
    if key not in _runners:
        _runners[key] = _Runner(reps=reps, **build_kw)
    return _runners[key]


# production configuration: AllReduce tail (measured cheaper on HW than
# ReduceScatter), reordered startup DMAs, early dummy-CC channel warmup
# (hides the tail collective's first-use init/sync under the conv loop)
PROD_KW = dict(
    reps=1, rs_tail=False, split_w=True, split_x0=True, lag=5, cc_warm=True
)


def kernel(x, w1, w2):
    in_maps = _prep_inputs(x, w1, w2)
    results = get_runner(**PROD_KW).run(in_maps)
    out = np.empty((B, COUT, CIN, 9), np.float32)
    for b in range(B):
        if PROD_KW.get("rs_tail"):
            out[b, : COUT // 2] = results[2 * b]["out"]
            out[b, COUT // 2 :] = results[2 * b + 1]["out"]
        else:
            out[b] = results[2 * b]["out"]
    return out

